# revision 19
# baseline (speedup 1.0000x reference)
"""Trainium2 Bass kernel for nn_MixedHeads (causal multi-head attention).

Reference computes, per (b, h):
  Q = x[b,:, :1024] @ Wq[h,:64,:1024].T      [T, 64]
  S = Q @ K.T * 0.125, causal mask, softmax
  O = P @ V, concat heads, pad to 2048 cols.

Sharding over 8 cores: core c -> batch b=c//2, heads h0=8*(c%2) .. h0+8.
Each core only reads its batch slice of x (8 MB) and its 8 heads' weights.

Device kernel v3 (same SPMD program on all cores, data differs per core):
  Weights (6 MB) are DMA'd to SBUF once per NEFF and stay resident.
  All engine streams execute in order, so the emission order IS the
  schedule; the kernel is software-pipelined at two levels:
    - projection chunks (per q-block) are interleaved with attention
      i-blocks: attn(*, i) only needs projections q <= i, so proj(q=i+1)
      emits between attention blocks and fills PE gaps left by exp waits;
    - within an (h, i) attention block, S^T pair-matmuls are emitted two
      pairs ahead of the PV matmuls that consume their exp, hiding the
      ACT latency behind PE work.
  Projection (fp32r): Q^T/K^T: out = Wchunk.T @ xsT -> [128=2hx64d, t],
    copied to bf16 SBUF on DVE. V: out = xsT.T @ Wv -> [t, 8hx64d], bf16,
    with a ones column per head so PV also emits the softmax denominator.
  Attention (bf16): S^T = K^T.T @ Q^T per 128-s-chunk, causal blocks only,
    diagonal blocks column-trimmed exactly; j-pairs share one [128,1024]
    PSUM tile; one full-width exp per pair (matmul start=True zeroes the
    2KB region, so trimmed-away columns read 0 -> exp gives unread 1s).
    P^T = exp(0.125*S^T + mask) -> pt bf16;  acc += V'.T @ P^T.
  acc [65, 512] -> ot -> DRAM; host divides rows 0..63 by row 64.
  PSUM: one [128,1024] pool (3 bufs) serves proj groups and S^T tiles,
  plus 2 accumulator banks = exactly 8 banks.
"""

import sys

sys.path.insert(0, "/opt/trn_rl_repo")

import numpy as np

import concourse.bass as bass
import concourse.tile as tile
from concourse import bacc, mybir
from concourse.bass_utils import run_bass_kernel_spmd

F32 = mybir.dt.float32
F32R = mybir.dt.float32r
BF16 = mybir.dt.bfloat16
EXP = mybir.ActivationFunctionType.Exp

B, TFULL, E, D = 4, 2048, 1024, 64
HPC = 8  # heads per core
NEG = -1.0e30
SCALE = 0.125


def build_nc(T=TFULL, reps=1, timing=False, loop_n=None):
    nq = T // 512   # q blocks
    ns = T // 128   # s chunks
    nc = bacc.Bacc(None, target_bir_lowering=False, enable_partition_id=False)
    xbt = nc.dram_tensor("xbt", [E, T], F32, kind="ExternalInput")
    wcat = nc.dram_tensor("wcat", [E, 3 * HPC * D], F32, kind="ExternalInput")
    maskd = nc.dram_tensor("maskd", [128, 128], F32, kind="ExternalInput")
    if timing:
        # timing builds keep all device work (incl. output DMAs) but point
        # them at internal DRAM so the tunnel moves no output bytes per call
        o = nc.dram_tensor("oint", [HPC, nq, 65, 512], F32, kind="Internal")
        t0 = nc.dram_tensor("t0", [1, 1], F32, kind="ExternalOutput")
    else:
        o = nc.dram_tensor("o", [HPC, nq, 65, 512], F32, kind="ExternalOutput")

    with tile.TileContext(nc) as tc:
        with (
            tc.tile_pool(name="const", bufs=1) as constp,
            tc.tile_pool(name="wpool", bufs=1) as wp,
            tc.tile_pool(name="qkstore", bufs=2) as qkp,
            tc.tile_pool(name="vstore", bufs=2) as vp,
            tc.tile_pool(name="xsT", bufs=2) as xtp,
            tc.tile_pool(name="ppool", bufs=3) as ptp,
            tc.tile_pool(name="ostage", bufs=2) as osp,
            tc.tile_pool(name="wkpsum", bufs=3, space="PSUM") as wkp,
            tc.tile_pool(name="accpsum", bufs=2, space="PSUM") as accp,
        ):
            mask = constp.tile([128, 128], F32, tag="mask")
            nc.sync.dma_start(mask[:], maskd[:])
            if timing:
                nc.sync.dma_start(t0[:], mask[0:1, 0:1])
            # weights resident in SBUF for the whole NEFF; per-e-chunk DMAs
            # give per-region deps so e=0 matmuls start after ~2.4us
            W = wp.tile([128, 8 * 1536], F32R, tag="w")
            for e in range(8):
                nc.sync.dma_start(
                    W[:, 1536 * e : 1536 * (e + 1)],
                    wcat[128 * e : 128 * (e + 1), :].bitcast(F32R),
                )


            def emit_proj_q(q, Qs, Ks, Vs, xsT):
                """Emission chunks for projecting q-block q: 12 closures."""
                chunks = []
                for g in range(8):
                    def qk_group(g=g):
                        pg = wkp.tile([128, 1024], F32, tag="sp")
                        for e in range(8):
                            nc.tensor.matmul(
                                pg[:, 0:512],
                                W[:, 1536 * e + 128 * g : 1536 * e + 128 * (g + 1)],
                                xsT[:, 512 * e : 512 * (e + 1)],
                                start=(e == 0),
                                stop=(e == 7),
                            )
                        dst = Qs if g < 4 else Ks
                        gg = g % 4
                        nc.vector.tensor_copy(
                            dst[:, T * gg + 512 * q : T * gg + 512 * (q + 1)],
                            pg[:, 0:512],
                        )
                    chunks.append(qk_group)
                for i in range(4):
                    def v_group(i=i):
                        pv = wkp.tile([128, 1024], F32, tag="sp")
                        for e in range(8):
                            nc.tensor.matmul(
                                pv[:, 0:512],
                                xsT[:, 512 * e + 128 * i : 512 * e + 128 * (i + 1)],
                                W[:, 1536 * e + 1024 : 1536 * e + 1536],
                                start=(e == 0),
                                stop=(e == 7),
                            )
                        c = 4 * q + i
                        nc.vector.tensor_copy(
                            Vs[:, 520 * c : 520 * c + 520].rearrange(
                                "p (h d) -> p h d", h=HPC
                            )[:, :, 0:64],
                            pv[:, 0:512].rearrange("p (h d) -> p h d", h=HPC),
                        )
                    chunks.append(v_group)
                return chunks

            LOOKAHEAD = 2  # S pairs emitted ahead of their PV consumers
            pv_queue = []

            def drain_pv(limit):
                while len(pv_queue) > limit:
                    pv_queue.pop(0)()

            def emit_attn_block(h, i, Qs, Ks, Vs):
                """Attention for (head h, q-block i). S^T pairs are emitted
                eagerly; their PV consumers go through a global queue with
                LOOKAHEAD pairs in flight, so the pipeline also spans block
                boundaries in the in-order PE stream."""
                row = 64 * (h % 2)
                cb = T * (h // 2)
                jmax = 4 * i + 3
                npairs = 2 * i + 2
                state = {}

                def emit_s(p):
                    spt = wkp.tile([128, 1024], F32, tag="sp")
                    pt = ptp.tile([128, 1024], BF16, tag="pt")
                    js = (2 * p, 2 * p + 1)
                    c0s = [128 * (j - 4 * i) if j >= 4 * i else 0 for j in js]
                    for k, (j, c0) in enumerate(zip(js, c0s)):
                        nc.tensor.matmul(
                            spt[:, 512 * k + c0 : 512 * (k + 1)],
                            Ks[row : row + 64, cb + 128 * j : cb + 128 * (j + 1)],
                            Qs[
                                row : row + 64,
                                cb + 512 * i + c0 : cb + 512 * (i + 1),
                            ],
                            start=True,
                            stop=True,
                        )
                        if j >= 4 * i:
                            nc.vector.tensor_add(
                                spt[:, 512 * k + c0 : 512 * k + c0 + 128],
                                spt[:, 512 * k + c0 : 512 * k + c0 + 128],
                                mask[:],
                            )
                    if c0s[1] == 0:
                        nc.scalar.activation(pt[:], spt[:], EXP, scale=SCALE)
                    else:
                        for k, c0 in enumerate(c0s):
                            nc.scalar.activation(
                                pt[:, 512 * k + c0 : 512 * (k + 1)],
                                spt[:, 512 * k + c0 : 512 * (k + 1)],
                                EXP,
                                scale=SCALE,
                            )
                    return pt, js, c0s

                def make_pv(p, s_out):
                    pt, js, c0s = s_out
                    last = p == npairs - 1

                    def pv():
                        if p == 0:
                            state["acc"] = accp.tile(
                                [128, 512], F32, tag="acc", name=f"acc{h}_{i}"
                            )
                        acc = state["acc"]
                        for k, (j, c0) in enumerate(zip(js, c0s)):
                            nc.tensor.matmul(
                                acc[0:65, c0:512],
                                Vs[:, 520 * j + 65 * h : 520 * j + 65 * h + 65],
                                pt[:, 512 * k + c0 : 512 * (k + 1)],
                                start=(j == 0),
                                stop=(j == jmax),
                            )
                        if last:
                            ot = osp.tile([128, 512], F32, tag="ot")
                            nc.vector.tensor_copy(ot[0:65, :], acc[0:65, :])
                            nc.sync.dma_start(o[h, i], ot[0:65, :])

                    return pv

                for p in range(npairs):
                    s_out = emit_s(p)
                    pv_queue.append(make_pv(p, s_out))
                    drain_pv(LOOKAHEAD)

            def emit_body():
                Qs = qkp.tile([128, 4 * T], BF16, tag="qs")
                Ks = qkp.tile([128, 4 * T], BF16, tag="ks")
                Vs = vp.tile([128, ns * 520], BF16, tag="vs")
                # ones column per (chunk, head) -> PV row 64 = denominator
                nc.gpsimd.memset(
                    Vs[:].rearrange("p (c h d) -> p c h d", h=HPC, d=65)[
                        :, :, :, 64:65
                    ],
                    1.0,
                )

                def start_proj(q):
                    xsT = xtp.tile([128, 8 * 512], F32R, tag="xst")
                    nc.sync.dma_start(
                        xsT[:].rearrange("p (ec c) -> p ec c", ec=8),
                        xbt[:, 512 * q : 512 * (q + 1)]
                        .bitcast(F32R)
                        .rearrange("(ec p) c -> p ec c", p=128),
                    )
                    return emit_proj_q(q, Qs, Ks, Vs, xsT)

                # prologue: q=0 fully projected
                for ch in start_proj(0):
                    ch()
                # attn(*, i) interleaved with proj(q=i+1)
                for i in range(nq):
                    pending = start_proj(i + 1) if i + 1 < nq else []
                    for h in range(HPC):
                        emit_attn_block(h, i, Qs, Ks, Vs)
                        # spread the 12 proj chunks across the 8 heads
                        take = (len(pending) + (HPC - 1 - h)) // (HPC - h)
                        for _ in range(take):
                            pending.pop(0)()
                drain_pv(0)

            if loop_n is not None:
                with tc.For_i(0, loop_n):
                    emit_body()
            else:
                for _rep in range(reps):
                    emit_body()

    nc.compile()
    return nc


def make_in_maps(x, Wq, Wk, Wv, T=TFULL):
    x = np.asarray(x, np.float32)
    mask = np.where(
        np.arange(128)[None, :] >= np.arange(128)[:, None], 0.0, NEG
    ).astype(np.float32)
    in_maps = []
    for c in range(8):
        b, h0 = c // 2, HPC * (c % 2)
        xbv = np.ascontiguousarray(x[b, :T, :E].T)  # [E, T]
        parts = []
        for Wg in (Wq, Wk, Wv):
            wg = np.asarray(Wg, np.float32)[h0 : h0 + HPC, :D, :E]  # [8, 64, 1024]
            parts.append(wg.transpose(2, 0, 1).reshape(E, HPC * D))
        wcat = np.ascontiguousarray(np.concatenate(parts, axis=1))  # [1024, 1536]
        in_maps.append({"xbt": xbv, "wcat": wcat, "maskd": mask})
    return in_maps


def assemble(results, T=TFULL):
    out = np.zeros((B, TFULL, 2048), np.float32)
    for c in range(8):
        b, h0 = c // 2, HPC * (c % 2)
        ov = np.asarray(results[c]["o"])  # [8, nq, 65, 512]
        On = ov[:, :, :64, :] / ov[:, :, 64:65, :]  # [8, nq, 64, 512]
        blk = On.transpose(1, 3, 0, 2).reshape(T, HPC * D)  # [(i f), (h d)]
        out[b, :T, D * h0 : D * h0 + HPC * D] = blk
    return out


def kernel(**inputs):
    nc = build_nc()
    in_maps = make_in_maps(inputs["x"], inputs["Wq"], inputs["Wk"], inputs["Wv"])
    res = run_bass_kernel_spmd(nc, in_maps, core_ids=list(range(8)))
    return assemble(res.results)


# revision 20
# speedup vs baseline: 1.0361x; 1.0361x over previous
"""Trainium2 Bass kernel for nn_MixedHeads (causal multi-head attention).

Reference computes, per (b, h):
  Q = x[b,:, :1024] @ Wq[h,:64,:1024].T      [T, 64]
  S = Q @ K.T * 0.125, causal mask, softmax
  O = P @ V, concat heads, pad to 2048 cols.

Sharding over 8 cores: core c -> batch b=c//2, heads h0=8*(c%2) .. h0+8.
Each core only reads its batch slice of x (8 MB) and its 8 heads' weights.

Device kernel v4 (same SPMD program on all cores, data differs per core):
  Weights (6 MB) are DMA'd to SBUF once per NEFF and stay resident.
  All engine streams execute in order, so the emission order IS the
  schedule; the kernel is software-pipelined at two levels:
    - projection chunks (per q-block) are interleaved with attention
      i-blocks: attn(*, i) only needs projections q <= i, so proj(q=i+1)
      emits between attention blocks and fills PE gaps left by exp waits;
    - S^T pair-matmuls are emitted ahead of the PV matmuls that consume
      their exp via a global 2-deep PV queue that also spans block
      boundaries, hiding the ACT latency behind PE work.
  Projection (fp32r): Q^T/K^T: out = Wchunk.T @ xsT -> [128=2hx64d, t],
    copied to bf16 SBUF on DVE. V: out = xsT.T @ Wv -> [t, 8hx64d], bf16,
    with a ones column per head so PV also emits the softmax denominator.
  Attention (bf16): S^T = K^T.T @ Q^T per 128-s-chunk, causal blocks only,
    diagonal blocks column-trimmed exactly (bf16 has no <256-col rate
    penalty); j-pairs share one [128,1024] PSUM tile; one exp per full
    pair, two exact-region exps per diagonal pair.
    P^T = exp(0.125*S^T + mask) -> pt bf16;  acc += V'.T @ P^T.
  acc [65, 512] -> ot -> DRAM; host divides rows 0..63 by row 64.
  PSUM: one [128,1024] pool (3 bufs) serves proj groups and S^T tiles,
  plus 2 accumulator banks = exactly 8 banks.
"""

import sys

sys.path.insert(0, "/opt/trn_rl_repo")

import numpy as np

import concourse.bass as bass
import concourse.tile as tile
from concourse import bacc, mybir
from concourse.bass_utils import run_bass_kernel_spmd

F32 = mybir.dt.float32
F32R = mybir.dt.float32r
BF16 = mybir.dt.bfloat16
EXP = mybir.ActivationFunctionType.Exp

B, TFULL, E, D = 4, 2048, 1024, 64
HPC = 8  # heads per core
NEG = -1.0e30
SCALE = 0.125


def build_nc(T=TFULL, reps=1, timing=False, loop_n=None):
    nq = T // 512   # q blocks
    ns = T // 128   # s chunks
    nc = bacc.Bacc(None, target_bir_lowering=False, enable_partition_id=False)
    xbt = nc.dram_tensor("xbt", [E, T], F32, kind="ExternalInput")
    wcat = nc.dram_tensor("wcat", [E, 3 * HPC * D], F32, kind="ExternalInput")
    maskd = nc.dram_tensor("maskd", [128, 128], F32, kind="ExternalInput")
    if timing:
        # timing builds keep all device work (incl. output DMAs) but point
        # them at internal DRAM so the tunnel moves no output bytes per call
        o = nc.dram_tensor("oint", [HPC, nq, 65, 512], F32, kind="Internal")
        t0 = nc.dram_tensor("t0", [1, 1], F32, kind="ExternalOutput")
    else:
        o = nc.dram_tensor("o", [HPC, nq, 65, 512], F32, kind="ExternalOutput")

    with tile.TileContext(nc) as tc:
        with (
            tc.tile_pool(name="const", bufs=1) as constp,
            tc.tile_pool(name="wpool", bufs=1) as wp,
            tc.tile_pool(name="qkstore", bufs=2) as qkp,
            tc.tile_pool(name="vstore", bufs=2) as vp,
            tc.tile_pool(name="xsT", bufs=2) as xtp,
            tc.tile_pool(name="ppool", bufs=3) as ptp,
            tc.tile_pool(name="ostage", bufs=2) as osp,
            tc.tile_pool(name="wkpsum", bufs=3, space="PSUM") as wkp,
            tc.tile_pool(name="accpsum", bufs=2, space="PSUM") as accp,
        ):
            mask = constp.tile([128, 128], F32, tag="mask")
            nc.sync.dma_start(mask[:], maskd[:])
            if timing:
                nc.sync.dma_start(t0[:], mask[0:1, 0:1])
            # weights resident in SBUF for the whole NEFF; per-e-chunk DMAs
            # give per-region deps so e=0 matmuls start after ~2.4us
            W = wp.tile([128, 8 * 1536], F32R, tag="w")
            for e in range(8):
                nc.sync.dma_start(
                    W[:, 1536 * e : 1536 * (e + 1)],
                    wcat[128 * e : 128 * (e + 1), :].bitcast(F32R),
                )


            def emit_proj_q(q, Qs, Ks, Vs, xsT):
                """Emission chunks for projecting q-block q: 12 closures."""
                chunks = []
                for g in range(8):
                    def qk_group(g=g):
                        pg = wkp.tile([128, 1024], F32, tag="sp")
                        for e in range(8):
                            nc.tensor.matmul(
                                pg[:, 0:512],
                                W[:, 1536 * e + 128 * g : 1536 * e + 128 * (g + 1)],
                                xsT[:, 512 * e : 512 * (e + 1)],
                                start=(e == 0),
                                stop=(e == 7),
                            )
                        dst = Qs if g < 4 else Ks
                        gg = g % 4
                        nc.vector.tensor_copy(
                            dst[:, T * gg + 512 * q : T * gg + 512 * (q + 1)],
                            pg[:, 0:512],
                        )
                    chunks.append(qk_group)
                for i in range(4):
                    def v_group(i=i):
                        pv = wkp.tile([128, 1024], F32, tag="sp")
                        for e in range(8):
                            nc.tensor.matmul(
                                pv[:, 0:512],
                                xsT[:, 512 * e + 128 * i : 512 * e + 128 * (i + 1)],
                                W[:, 1536 * e + 1024 : 1536 * e + 1536],
                                start=(e == 0),
                                stop=(e == 7),
                            )
                        c = 4 * q + i
                        nc.vector.tensor_copy(
                            Vs[:, 520 * c : 520 * c + 520].rearrange(
                                "p (h d) -> p h d", h=HPC
                            )[:, :, 0:64],
                            pv[:, 0:512].rearrange("p (h d) -> p h d", h=HPC),
                        )
                    chunks.append(v_group)
                return chunks

            LOOKAHEAD = 2  # S pairs emitted ahead of their PV consumers
            pv_queue = []

            def drain_pv(limit):
                while len(pv_queue) > limit:
                    pv_queue.pop(0)()

            def emit_attn_block(h, i, Qs, Ks, Vs):
                """Attention for (head h, q-block i). S^T pairs are emitted
                eagerly; their PV consumers go through a global queue with
                LOOKAHEAD pairs in flight, so the pipeline also spans block
                boundaries in the in-order PE stream."""
                row = 64 * (h % 2)
                cb = T * (h // 2)
                jmax = 4 * i + 3
                npairs = 2 * i + 2
                state = {}

                def emit_s(p):
                    spt = wkp.tile([128, 1024], F32, tag="sp")
                    pt = ptp.tile([128, 1024], BF16, tag="pt")
                    js = (2 * p, 2 * p + 1)
                    c0s = [128 * (j - 4 * i) if j >= 4 * i else 0 for j in js]
                    for k, (j, c0) in enumerate(zip(js, c0s)):
                        nc.tensor.matmul(
                            spt[:, 512 * k + c0 : 512 * (k + 1)],
                            Ks[row : row + 64, cb + 128 * j : cb + 128 * (j + 1)],
                            Qs[
                                row : row + 64,
                                cb + 512 * i + c0 : cb + 512 * (i + 1),
                            ],
                            start=True,
                            stop=True,
                        )
                        if j >= 4 * i:
                            nc.vector.tensor_add(
                                spt[:, 512 * k + c0 : 512 * k + c0 + 128],
                                spt[:, 512 * k + c0 : 512 * k + c0 + 128],
                                mask[:],
                            )
                    if c0s[1] == 0:
                        nc.scalar.activation(pt[:], spt[:], EXP, scale=SCALE)
                    else:
                        for k, c0 in enumerate(c0s):
                            nc.scalar.activation(
                                pt[:, 512 * k + c0 : 512 * (k + 1)],
                                spt[:, 512 * k + c0 : 512 * (k + 1)],
                                EXP,
                                scale=SCALE,
                            )
                    return pt, js, c0s

                def make_pv(p, s_out):
                    pt, js, c0s = s_out
                    last = p == npairs - 1

                    def pv():
                        if p == 0:
                            state["acc"] = accp.tile(
                                [128, 512], F32, tag="acc", name=f"acc{h}_{i}"
                            )
                        acc = state["acc"]
                        for k, (j, c0) in enumerate(zip(js, c0s)):
                            nc.tensor.matmul(
                                acc[0:65, c0:512],
                                Vs[:, 520 * j + 65 * h : 520 * j + 65 * h + 65],
                                pt[:, 512 * k + c0 : 512 * (k + 1)],
                                start=(j == 0),
                                stop=(j == jmax),
                            )
                        if last:
                            ot = osp.tile([128, 512], F32, tag="ot")
                            nc.vector.tensor_copy(ot[0:65, :], acc[0:65, :])
                            nc.sync.dma_start(o[h, i], ot[0:65, :])

                    return pv

                for p in range(npairs):
                    s_out = emit_s(p)
                    pv_queue.append(make_pv(p, s_out))
                    drain_pv(LOOKAHEAD)

            def emit_body():
                Qs = qkp.tile([128, 4 * T], BF16, tag="qs")
                Ks = qkp.tile([128, 4 * T], BF16, tag="ks")
                Vs = vp.tile([128, ns * 520], BF16, tag="vs")
                # ones column per (chunk, head) -> PV row 64 = denominator
                nc.gpsimd.memset(
                    Vs[:].rearrange("p (c h d) -> p c h d", h=HPC, d=65)[
                        :, :, :, 64:65
                    ],
                    1.0,
                )

                def start_proj(q):
                    xsT = xtp.tile([128, 8 * 512], F32R, tag="xst")
                    nc.sync.dma_start(
                        xsT[:].rearrange("p (ec c) -> p ec c", ec=8),
                        xbt[:, 512 * q : 512 * (q + 1)]
                        .bitcast(F32R)
                        .rearrange("(ec p) c -> p ec c", p=128),
                    )
                    return emit_proj_q(q, Qs, Ks, Vs, xsT)

                # prologue: q=0 fully projected
                for ch in start_proj(0):
                    ch()
                # attn(*, i) interleaved with proj(q=i+1)
                for i in range(nq):
                    pending = start_proj(i + 1) if i + 1 < nq else []
                    for h in range(HPC):
                        emit_attn_block(h, i, Qs, Ks, Vs)
                        # spread the 12 proj chunks across the 8 heads
                        take = (len(pending) + (HPC - 1 - h)) // (HPC - h)
                        for _ in range(take):
                            pending.pop(0)()
                drain_pv(0)

            if loop_n is not None:
                with tc.For_i(0, loop_n):
                    emit_body()
            else:
                for _rep in range(reps):
                    emit_body()

    nc.compile()
    return nc


def make_in_maps(x, Wq, Wk, Wv, T=TFULL):
    x = np.asarray(x, np.float32)
    mask = np.where(
        np.arange(128)[None, :] >= np.arange(128)[:, None], 0.0, NEG
    ).astype(np.float32)
    in_maps = []
    for c in range(8):
        b, h0 = c // 2, HPC * (c % 2)
        xbv = np.ascontiguousarray(x[b, :T, :E].T)  # [E, T]
        parts = []
        for Wg in (Wq, Wk, Wv):
            wg = np.asarray(Wg, np.float32)[h0 : h0 + HPC, :D, :E]  # [8, 64, 1024]
            parts.append(wg.transpose(2, 0, 1).reshape(E, HPC * D))
        wcat = np.ascontiguousarray(np.concatenate(parts, axis=1))  # [1024, 1536]
        in_maps.append({"xbt": xbv, "wcat": wcat, "maskd": mask})
    return in_maps


def assemble(results, T=TFULL):
    out = np.zeros((B, TFULL, 2048), np.float32)
    for c in range(8):
        b, h0 = c // 2, HPC * (c % 2)
        ov = np.asarray(results[c]["o"])  # [8, nq, 65, 512]
        On = ov[:, :, :64, :] / ov[:, :, 64:65, :]  # [8, nq, 64, 512]
        blk = On.transpose(1, 3, 0, 2).reshape(T, HPC * D)  # [(i f), (h d)]
        out[b, :T, D * h0 : D * h0 + HPC * D] = blk
    return out


def kernel(**inputs):
    nc = build_nc()
    in_maps = make_in_maps(inputs["x"], inputs["Wq"], inputs["Wk"], inputs["Wv"])
    res = run_bass_kernel_spmd(nc, in_maps, core_ids=list(range(8)))
    return assemble(res.results)


# revision 25
# speedup vs baseline: 1.1904x; 1.1490x over previous
"""Trainium2 Bass kernel for nn_MixedHeads (causal multi-head attention).

Reference computes, per (b, h):
  Q = x[b,:, :1024] @ Wq[h,:64,:1024].T      [T, 64]
  S = Q @ K.T * 0.125, causal mask, softmax
  O = P @ V, concat heads, pad to 2048 cols.

Sharding over 8 cores: core c -> batch b=c//2, heads h0=8*(c%2) .. h0+8.
Each core only reads its batch slice of x (8 MB) and its 8 heads' weights.

Device kernel v4 (same SPMD program on all cores, data differs per core):
  Weights (6 MB) are DMA'd to SBUF once per NEFF and stay resident.
  All engine streams execute in order, so the emission order IS the
  schedule; the kernel is software-pipelined at two levels:
    - projection chunks (per q-block) are interleaved with attention
      i-blocks: attn(*, i) only needs projections q <= i, so proj(q=i+1)
      emits between attention blocks and fills PE gaps left by exp waits;
    - S^T pair-matmuls are emitted ahead of the PV matmuls that consume
      their exp via a global 2-deep PV queue that also spans block
      boundaries, hiding the ACT latency behind PE work.
  Projection (fp32r): Q^T/K^T: out = Wchunk.T @ xsT -> [128=2hx64d, t],
    copied to bf16 SBUF on DVE. V: out = xsT.T @ Wv -> [t, 8hx64d], bf16,
    with a ones column per head so PV also emits the softmax denominator.
  Attention (bf16): S^T = K^T.T @ Q^T per 128-s-chunk, causal blocks only,
    diagonal blocks column-trimmed exactly (bf16 has no <256-col rate
    penalty); j-pairs share one [128,1024] PSUM tile; one exp per full
    pair, two exact-region exps per diagonal pair.
    P^T = exp(0.125*S^T + mask) -> pt bf16;  acc += V'.T @ P^T.
  acc [65, 512] -> ot -> DRAM; host divides rows 0..63 by row 64.
  PSUM: one [128,1024] pool (3 bufs) serves proj groups and S^T tiles,
  plus 2 accumulator banks = exactly 8 banks.
"""

import sys

sys.path.insert(0, "/opt/trn_rl_repo")

import numpy as np

import concourse.bass as bass
import concourse.tile as tile
from concourse import bacc, mybir
from concourse.bass_utils import run_bass_kernel_spmd

F32 = mybir.dt.float32
F32R = mybir.dt.float32r
BF16 = mybir.dt.bfloat16
EXP = mybir.ActivationFunctionType.Exp

B, TFULL, E, D = 4, 2048, 1024, 64
HPC = 8  # heads per core
NEG = -1.0e30
SCALE = 0.125


def build_nc(T=TFULL, reps=1, timing=False, loop_n=None):
    nq = T // 512   # q blocks
    ns = T // 128   # s chunks
    nc = bacc.Bacc(None, target_bir_lowering=False, enable_partition_id=False)
    xbt = nc.dram_tensor("xbt", [E, T], F32, kind="ExternalInput")
    wcat = nc.dram_tensor("wcat", [E, 3 * HPC * D], F32, kind="ExternalInput")
    maskd = nc.dram_tensor("maskd", [128, 128], F32, kind="ExternalInput")
    mask01d = nc.dram_tensor("mask01d", [128, 128], F32, kind="ExternalInput")
    if timing:
        # timing builds keep all device work (incl. output DMAs) but point
        # them at internal DRAM so the tunnel moves no output bytes per call
        o = nc.dram_tensor("oint", [HPC, nq, 65, 512], F32, kind="Internal")
        t0 = nc.dram_tensor("t0", [1, 1], F32, kind="ExternalOutput")
    else:
        o = nc.dram_tensor("o", [HPC, nq, 65, 512], F32, kind="ExternalOutput")

    with tile.TileContext(nc) as tc:
        with (
            tc.tile_pool(name="const", bufs=1) as constp,
            tc.tile_pool(name="wpool", bufs=1) as wp,
            tc.tile_pool(name="qkstore", bufs=2) as qkp,
            tc.tile_pool(name="vstore", bufs=2) as vp,
            tc.tile_pool(name="xsT", bufs=2) as xtp,
            tc.tile_pool(name="ppool", bufs=3) as ptp,
            tc.tile_pool(name="ostage", bufs=2) as osp,
            tc.tile_pool(name="wkpsum", bufs=3, space="PSUM") as wkp,
            tc.tile_pool(name="accpsum", bufs=2, space="PSUM") as accp,
        ):
            mask = constp.tile([128, 128], F32, tag="mask")
            nc.sync.dma_start(mask[:], maskd[:])
            m01 = constp.tile([128, 128], F32, tag="m01")
            nc.sync.dma_start(m01[:], mask01d[:])
            if timing:
                nc.sync.dma_start(t0[:], mask[0:1, 0:1])
            # weights resident in SBUF for the whole NEFF; per-e-chunk DMAs
            # give per-region deps so e=0 matmuls start after ~2.4us
            W = wp.tile([128, 8 * 1536], F32R, tag="w")
            for e in range(8):
                nc.sync.dma_start(
                    W[:, 1536 * e : 1536 * (e + 1)],
                    wcat[128 * e : 128 * (e + 1), :].bitcast(F32R),
                )


            def emit_proj_q(q, Qs, Ks, Vs, xsT):
                """Emission chunks for projecting q-block q: 12 closures."""
                chunks = []
                for g in range(8):
                    def qk_group(g=g):
                        pg = wkp.tile([128, 1024], F32, tag="sp")
                        for e in range(8):
                            nc.tensor.matmul(
                                pg[:, 0:512],
                                W[:, 1536 * e + 128 * g : 1536 * e + 128 * (g + 1)],
                                xsT[:, 512 * e : 512 * (e + 1)],
                                start=(e == 0),
                                stop=(e == 7),
                            )
                        dst = Qs if g < 4 else Ks
                        gg = g % 4
                        nc.vector.tensor_copy(
                            dst[:, T * gg + 512 * q : T * gg + 512 * (q + 1)],
                            pg[:, 0:512],
                        )
                    chunks.append(qk_group)
                for i in range(4):
                    def v_group(i=i):
                        pv = wkp.tile([128, 1024], F32, tag="sp")
                        for e in range(8):
                            nc.tensor.matmul(
                                pv[:, 0:512],
                                xsT[:, 512 * e + 128 * i : 512 * e + 128 * (i + 1)],
                                W[:, 1536 * e + 1024 : 1536 * e + 1536],
                                start=(e == 0),
                                stop=(e == 7),
                            )
                        c = 4 * q + i
                        nc.vector.tensor_copy(
                            Vs[:, 520 * c : 520 * c + 520].rearrange(
                                "p (h d) -> p h d", h=HPC
                            )[:, :, 0:64],
                            pv[:, 0:512].rearrange("p (h d) -> p h d", h=HPC),
                        )
                    chunks.append(v_group)
                return chunks

            LOOKAHEAD = 2  # S pairs emitted ahead of their PV consumers
            pv_queue = []

            def drain_pv(limit):
                while len(pv_queue) > limit:
                    pv_queue.pop(0)()

            def emit_attn_block(h, i, Qs, Ks, Vs):
                """Attention for (head h, q-block i). S^T pairs are emitted
                eagerly; their PV consumers go through a global queue with
                LOOKAHEAD pairs in flight, so the pipeline also spans block
                boundaries in the in-order PE stream."""
                row = 64 * (h % 2)
                cb = T * (h // 2)
                jmax = 4 * i + 3
                npairs = 2 * i + 2
                state = {}

                def emit_s(p):
                    spt = wkp.tile([128, 1024], F32, tag="sp")
                    pt = ptp.tile([128, 1024], BF16, tag="pt")
                    js = (2 * p, 2 * p + 1)
                    c0s = [128 * (j - 4 * i) if j >= 4 * i else 0 for j in js]
                    for k, (j, c0) in enumerate(zip(js, c0s)):
                        nc.tensor.matmul(
                            spt[:, 512 * k + c0 : 512 * (k + 1)],
                            Ks[row : row + 64, cb + 128 * j : cb + 128 * (j + 1)],
                            Qs[
                                row : row + 64,
                                cb + 512 * i + c0 : cb + 512 * (i + 1),
                            ],
                            start=True,
                            stop=True,
                        )
                    if c0s[1] == 0:
                        nc.scalar.activation(pt[:], spt[:], EXP, scale=SCALE)
                    else:
                        for k, c0 in enumerate(c0s):
                            nc.scalar.activation(
                                pt[:, 512 * k + c0 : 512 * (k + 1)],
                                spt[:, 512 * k + c0 : 512 * (k + 1)],
                                EXP,
                                scale=SCALE,
                            )
                    for k, (j, c0) in enumerate(zip(js, c0s)):
                        if j >= 4 * i:
                            nc.vector.tensor_mul(
                                pt[:, 512 * k + c0 : 512 * k + c0 + 128],
                                pt[:, 512 * k + c0 : 512 * k + c0 + 128],
                                m01[:],
                            )
                    return pt, js, c0s

                def make_pv(p, s_out):
                    pt, js, c0s = s_out
                    last = p == npairs - 1

                    def pv():
                        if p == 0:
                            state["acc"] = accp.tile(
                                [128, 512], F32, tag="acc", name=f"acc{h}_{i}"
                            )
                        acc = state["acc"]
                        for k, (j, c0) in enumerate(zip(js, c0s)):
                            nc.tensor.matmul(
                                acc[0:65, c0:512],
                                Vs[:, 520 * j + 65 * h : 520 * j + 65 * h + 65],
                                pt[:, 512 * k + c0 : 512 * (k + 1)],
                                start=(j == 0),
                                stop=(j == jmax),
                            )
                        if last:
                            ot = osp.tile([128, 512], F32, tag="ot")
                            nc.vector.tensor_copy(ot[0:65, :], acc[0:65, :])
                            nc.sync.dma_start(o[h, i], ot[0:65, :])

                    return pv

                for p in range(npairs):
                    s_out = emit_s(p)
                    pv_queue.append(make_pv(p, s_out))
                    drain_pv(LOOKAHEAD)

            def emit_body():
                Qs = qkp.tile([128, 4 * T], BF16, tag="qs")
                Ks = qkp.tile([128, 4 * T], BF16, tag="ks")
                Vs = vp.tile([128, ns * 520], BF16, tag="vs")
                # ones column per (chunk, head) -> PV row 64 = denominator
                nc.gpsimd.memset(
                    Vs[:].rearrange("p (c h d) -> p c h d", h=HPC, d=65)[
                        :, :, :, 64:65
                    ],
                    1.0,
                )

                def start_proj(q):
                    xsT = xtp.tile([128, 8 * 512], F32R, tag="xst")
                    nc.sync.dma_start(
                        xsT[:].rearrange("p (ec c) -> p ec c", ec=8),
                        xbt[:, 512 * q : 512 * (q + 1)]
                        .bitcast(F32R)
                        .rearrange("(ec p) c -> p ec c", p=128),
                    )
                    return emit_proj_q(q, Qs, Ks, Vs, xsT)

                # prologue: q=0 fully projected
                for ch in start_proj(0):
                    ch()
                # attn(*, i) interleaved with proj(q=i+1)
                for i in range(nq):
                    pending = start_proj(i + 1) if i + 1 < nq else []
                    for h in range(HPC):
                        emit_attn_block(h, i, Qs, Ks, Vs)
                        # spread the 12 proj chunks across the 8 heads
                        take = (len(pending) + (HPC - 1 - h)) // (HPC - h)
                        for _ in range(take):
                            pending.pop(0)()
                drain_pv(0)

            if loop_n is not None:
                with tc.For_i(0, loop_n):
                    emit_body()
            else:
                for _rep in range(reps):
                    emit_body()

    nc.compile()
    return nc


def make_in_maps(x, Wq, Wk, Wv, T=TFULL):
    x = np.asarray(x, np.float32)
    mask = np.where(
        np.arange(128)[None, :] >= np.arange(128)[:, None], 0.0, NEG
    ).astype(np.float32)
    mask01 = (np.arange(128)[None, :] >= np.arange(128)[:, None]).astype(np.float32)
    in_maps = []
    for c in range(8):
        b, h0 = c // 2, HPC * (c % 2)
        xbv = np.ascontiguousarray(x[b, :T, :E].T)  # [E, T]
        parts = []
        for Wg in (Wq, Wk, Wv):
            wg = np.asarray(Wg, np.float32)[h0 : h0 + HPC, :D, :E]  # [8, 64, 1024]
            parts.append(wg.transpose(2, 0, 1).reshape(E, HPC * D))
        wcat = np.ascontiguousarray(np.concatenate(parts, axis=1))  # [1024, 1536]
        in_maps.append(
            {"xbt": xbv, "wcat": wcat, "maskd": mask, "mask01d": mask01}
        )
    return in_maps


def assemble(results, T=TFULL):
    out = np.zeros((B, TFULL, 2048), np.float32)
    for c in range(8):
        b, h0 = c // 2, HPC * (c % 2)
        ov = np.asarray(results[c]["o"])  # [8, nq, 65, 512]
        On = ov[:, :, :64, :] / ov[:, :, 64:65, :]  # [8, nq, 64, 512]
        blk = On.transpose(1, 3, 0, 2).reshape(T, HPC * D)  # [(i f), (h d)]
        out[b, :T, D * h0 : D * h0 + HPC * D] = blk
    return out


def kernel(**inputs):
    nc = build_nc()
    in_maps = make_in_maps(inputs["x"], inputs["Wq"], inputs["Wk"], inputs["Wv"])
    res = run_bass_kernel_spmd(nc, in_maps, core_ids=list(range(8)))
    return assemble(res.results)
